# revision 65
# baseline (speedup 1.0000x reference)
"""Trainium2 Bass kernel for single-head full-softmax attention.

Reference computation (B=4, T=4096, D=768, H=64):
    Q = x @ Wq.T + bq ; K = x @ Wk.T + bk ; V = x @ Wv.T + bv
    out = softmax(Q K^T / 8) @ V          (no causal mask)

Sharding: 8 cores; core i owns batch b=i//2, token half i%2 (2048 tokens).
Each core projects Q/K/V for its own tokens; K/V halves are exchanged
within core pairs {2b, 2b+1} via AllGather, and each core runs attention
for its 2048 queries against the full 4096 keys (local keys first).

Key choices vs the naive formulation (cost model: matmul cost =
output-free-size x cycles-per-row, fp8 DoubleRow 0.5 cyc/row; exp
throughput bound by the ACT/DVE PSUM->SBUF element rates; one serial
DMA device with a ~630ns HWDGE surcharge per DMA instruction):

  - x ships as two fp8 streams: x8 (value) first, ex8 (residual) after.
    Q/K project from x8 only with both weight slots (W8*x8 + eW8*x8 =
    W*x8): 6 DR matmuls per chunk-output instead of 9; the ~3% x-quant
    error on Q/K costs ~4e-3 extra output err (budget 2e-2).  V needs
    full x precision (V error passes straight to the output), so V
    blocks keep 3 terms and start once their ex8 chunk lands.
  - QK^T runs in fp8e4m3 DoubleRow: lhsT = [K8^T tile | zeros],
    rhs = [Q8 | zeros] (zero slots written once by gpsimd memsets).
  - P = exp(S) stays bf16.  The exp work is split between the
    Activation engine (true exp) and the DVE, which computes a
    bias-calibrated Schraudolph exp in ONE tensor_scalar op:
    bf16(P) = bitcast_int16(round(S * 128*log2e/8 + 16248.7)).
  - P@V is emitted with P^T tiles as the *stationary* operand so each
    accumulating matmul outputs [128q, 65] (cost 65 rows) instead of
    [65, 512] (cost 512), landing in [token, h] layout.
  - V1 = [V | 1] ones-column (gpsimd memset) yields the softmax
    denominator inside the same PV accumulation; the final num/den
    division AND the V bias (softmax-invariant: out = num/den + bv)
    happen on the host.
  - Aux work is kept off the exp-critical ACT/DVE engines: DR zero
    slots + ones column via gpsimd memsets; chunks without a local-PV
    stage DMA their PSUM accumulator straight to DRAM (no copy).
  - The ACT exp table load (~1.3us) is triggered by a 1-element exp at
    kernel start so it overlaps the input DMA, and a few large PE
    warm-up matmuls bridge the p-state ramp until the first x8 chunk
    lands.
  - Schedule: triangular local phase paced by the x8 chunk DMAs, K and
    V exchanged in two separate early collectives; then chunk-major
    remote phase with the previous chunk's PV matmuls interleaved; the
    final chunk's PV is split across two PSUM accumulators so only ~80
    matmuls trail the last exp tile.
"""

import numpy as np
import ml_dtypes

import concourse.bass as bass
import concourse.tile as tile
from concourse import bacc, mybir
from concourse.bass import ts, ds
from concourse.bass_utils import run_bass_kernel_spmd

BF16 = mybir.dt.bfloat16
F32 = mybir.dt.float32
F8 = mybir.dt.float8e4
I16 = mybir.dt.int16
U8 = mybir.dt.uint8
U32 = mybir.dt.uint32

B, T, D, H = 4, 4096, 768, 64
H1 = H + 1          # V augmented with ones column
NCORES = 8
TL = T // 2         # 2048 local tokens / queries per core
DT = D // 128       # 6 d-tiles
KT = T // 128       # 32 k-tiles over the full sequence
KTL = TL // 128     # 16 k-tiles per half
QC = TL // 512      # 4 query chunks of 512
SCALE = 1.0 / 8.0   # 1/sqrt(64)
WCOLS = 64 + 64 + H1     # packed weight columns (wqT | wkT | wv1)
WCOLS_P = 208            # padded to a 16B-aligned DoubleRow slot stride

LOG2E = 1.4426950408889634
A_SCH = SCALE * 128.0 * LOG2E      # Schraudolph scale (fold in 1/8)
B_SCH = 16256.0 - 7.3              # exponent bias + mean-bias calibration

K_BYTES = 64 * TL          # fp8 K^T payload bytes
V_BYTES = TL * H1 * 2      # bf16 V1 payload bytes

REPLICA_GROUPS = [[0, 1], [2, 3], [4, 5], [6, 7]]
EXP = mybir.ActivationFunctionType.Exp
IDENT = mybir.ActivationFunctionType.Identity
DR = mybir.MatmulPerfMode.DoubleRow

# modeled per-op engine costs (ns) for the greedy exp-assignment balance
ACT_EXP_NS = 1038.0
DVE_EXP_NS = 1192.0
ACT_Q8_NS = 612.0
DVE_K8_NS = 658.0
DVE_V_NS = 396.0
OUT_NS = 400.0
DVE_BIAS = 0.0      # >0 biases exp assignment away from DVE
# engine assignment for the 64 score-pair exps (A=ACT, D=DVE Schraudolph),
# found by local search under TimelineSim
EXP_PATTERN = [ch == 'A' for ch in
               "DADDADDADDADADADDAADADADADADADADADADADADADAADADADADADADAADADADAD"]
ASSIGN_LOG = []     # realized assignment of the last build
K8_ENG = "scalar"   # engine for the K fp8 copies
V_ENG = "scalar"    # engine for the V copies

WARM_N = 6          # PE warm-up matmuls ([64, 512] each)


def build_body(nc, tc, ap, psum, sbuf, fake_collective=False):
    """Emit one full forward pass. ap: dict of DRAM APs."""

    # ---- x^T fp8 value/residual slots; x8 chunks ship first so Q/K
    # projections (x8-only) start ~1us after the weights land, ex8
    # chunks follow for the V projections ----
    x2_sb = sbuf.tile([128, DT, 2, TL], F8, tag="x2", bufs=1)

    def emit_x_chunk(c, xs, split=False):
        src = ap["xT8"] if xs == 0 else ap["xTe"]
        if split:
            # two DMAs so the first projection matmuls (subtile deps)
            # start ~0.7us before the full chunk lands
            nc.sync.dma_start(
                out=x2_sb[:, 0:4, xs, ts(c, 512)],
                in_=src[:, c, 0:2].rearrange("p j d t -> p (j d) t"))
            nc.sync.dma_start(
                out=x2_sb[:, 4:6, xs, ts(c, 512)],
                in_=src[:, c, 2:3].rearrange("p j d t -> p (j d) t"))
        else:
            nc.sync.dma_start(
                out=x2_sb[:, :, xs, ts(c, 512)],
                in_=src[:, c].rearrange("p j d t -> p (j d) t"))

    # ---- packed fp8 weights (32x scaled, + scaled residual slot), first
    # in the priority-ordered single DMA queue ----
    wpack_sb = sbuf.tile([128, 2, DT, WCOLS_P], F8, tag="wpack", bufs=1)
    bq_sb = sbuf.tile([128, 1], F32, tag="bq", bufs=1)
    # warm_sb memset FIRST on the gpsimd queue: the PE warm-up stream
    # and the ACT table-load prewarm both wait on it.
    warm_sb = sbuf.tile([128, 512], BF16, tag="warm", bufs=1)
    nc.gpsimd.memset(warm_sb, 0.0)
    # K/Q weight columns first (they gate the first projection); V
    # columns ride later in the queue (first V block runs ~10us in).
    wpack_ap = ap["wpack"].rearrange("p (s i h) -> p s i h", s=2, i=DT)
    nc.sync.dma_start(out=wpack_sb[:, :, :, 0:128], in_=wpack_ap[:, :, :, 0:128])
    emit_x_chunk(0, 0, split=True)
    nc.gpsimd.dma_start(out=bq_sb, in_=ap["bq"])

    # K^T/Q^T fp8; zero DoubleRow slots + V ones column via gpsimd
    # memsets (SBUF-only work, keeps ACT/DVE free for exp)
    k8_sb = sbuf.tile([64, 2, T], F8, tag="k8", bufs=1)
    q8_sb = sbuf.tile([64, 2, TL], F8, tag="q8", bufs=1)
    v1_sb = sbuf.tile([128, KT, H1], BF16, tag="v1", bufs=1)
    nc.gpsimd.memset(k8_sb[:, 1, :].bitcast(U32), 0)
    nc.gpsimd.memset(q8_sb[:, 1, :].bitcast(U32), 0)
    nc.gpsimd.memset(v1_sb[:, 0:KTL, H:H1], 1.0)

    for c in range(1, QC):
        emit_x_chunk(c, 0)
    nc.sync.dma_start(out=wpack_sb[:, :, :, 128:WCOLS_P],
                      in_=wpack_ap[:, :, :, 128:WCOLS_P])
    for c in range(QC):
        emit_x_chunk(c, 1)

    # ACT exp-table prewarm: the first Activation triggers a ~1.3us
    # LoadActFuncSet; fire it on 1 element right after warm_sb memsets
    # so it overlaps the input DMA instead of the first real exp.
    actwarm_sb = sbuf.tile([128, 1], BF16, tag="actwarm", bufs=1)
    nc.scalar.activation(out=actwarm_sb[0:1, :], in_=warm_sb[0:1, 0:1],
                         func=EXP, scale=1.0)

    # PE warm-up during the initial DMA wait: keeps the PE instruction
    # stream occupied past the p-state ramp so the projections are
    # costed at full clock; few large matmuls so the stream ends right
    # as the first x8 chunk lands.
    wps = psum.tile([128, 2, 512], F32, tag="st", bufs=3, name="wps")
    for _ in range(WARM_N):
        nc.tensor.matmul(wps[0:64, 0, :], warm_sb[:, 0:64],
                         warm_sb, start=True, stop=True)

    # DRAM bounce buffers for the pair exchange
    dram_cm = tc.tile_pool(name="dram", bufs=1, space="DRAM")
    dram = dram_cm.__enter__()
    bounce_k_in = dram.tile([K_BYTES], U8)
    bounce_k_out = dram.tile([2, K_BYTES], U8)
    bounce_v_in = dram.tile([2, V_BYTES // 2], U8)
    bounce_v_out = dram.tile([2, 2, V_BYTES // 2], U8)

    # ---- projections ----
    # Q/K from x8 only, both weight slots: (W8 + eW8) * x8 = W * x8.
    # V needs full x precision: 3 terms (W8*x8, W8*ex8, eW8*x8).
    # The 1/32 weight prescale is folded into the PSUM->SBUF copies.
    PROJ_TERMS_KQ = [(0, 0), (1, 0)]         # (w slot, x slot)
    PROJ_TERMS_V = [(0, 0), (0, 1), (1, 0)]

    def emit_kq_chunk(c):
        kqt = psum.tile([128, 4, 128], F32, tag="acc65", bufs=2, name=f"kq{c}")
        kq2 = psum.tile([128, 4, 128], F32, tag="acc65", bufs=2, name=f"kq2_{c}")
        kslc = kqt[0:64].rearrange("p a b -> p (a b)")
        qslc = kq2[0:64].rearrange("p a b -> p (a b)")
        for cols, oslc in ((slice(64, 128), kslc), (slice(0, 64), qslc)):
            n = 0
            for j in range(DT // 2):
                for ws, xs in PROJ_TERMS_KQ:
                    nc.tensor.matmul(
                        oslc, wpack_sb[:, ws, ds(2 * j, 2), cols],
                        x2_sb[:, ds(2 * j, 2), xs, ts(c, 512)],
                        start=(n == 0), stop=(n == 5), perf_mode=DR)
                    n += 1
        if K8_ENG == "vector" or c == 0:
            nc.vector.tensor_scalar_mul(k8_sb[:, 0, ts(c, 512)], kslc,
                                        1.0 / 32.0)
            eng_t["dve"] += DVE_K8_NS
        else:
            nc.scalar.activation(out=k8_sb[:, 0, ts(c, 512)], in_=kslc,
                                 func=IDENT, scale=1.0 / 32.0)
            eng_t["act"] += DVE_K8_NS
        nc.scalar.activation(out=q8_sb[:, 0, ts(c, 512)], in_=qslc,
                             func=IDENT, scale=1.0 / 32.0, bias=bq_sb[0:64, :])
        eng_t["act"] += ACT_Q8_NS

    def emit_v_block(r):
        vp = psum.tile([128, 4, 128], F32, tag="acc65", bufs=2, name=f"vp{r}")
        for t4 in range(4):
            t = 4 * r + t4
            n = 0
            for j in range(DT // 2):
                for ws, xs in PROJ_TERMS_V:
                    nc.tensor.matmul(
                        vp[:, t4, 0:H],
                        x2_sb[:, ds(2 * j, 2), xs, ts(t, 128)],
                        wpack_sb[:, ws, ds(2 * j, 2), 128:128 + H],
                        start=(n == 0), stop=(n == 8), perf_mode=DR)
                    n += 1
        if V_ENG == "vector":
            nc.vector.tensor_scalar_mul(v1_sb[:, ds(4 * r, 4), 0:H],
                                        vp[:, :, 0:H], 1.0 / 32.0)
            eng_t["dve"] += DVE_V_NS
        else:
            nc.scalar.activation(out=v1_sb[:, ds(4 * r, 4), 0:H],
                                 in_=vp[:, :, 0:H], func=IDENT,
                                 scale=1.0 / 32.0)
            eng_t["act"] += DVE_V_NS

    # ---- pair exchange, split: K8 first (feeds remote scores), V1 later ----
    def emit_exchange_k():
        nc.sync.dma_start(
            out=bounce_k_in.rearrange("(p t) -> p t", p=64),
            in_=k8_sb[:, 0, 0:TL].bitcast(U8))
        if fake_collective:
            nc.sync.dma_start(out=bounce_k_out[0], in_=bounce_k_in)
            nc.sync.dma_start(out=bounce_k_out[1], in_=bounce_k_in)
        else:
            nc.gpsimd.collective_compute(
                "AllGather", mybir.AluOpType.bypass,
                replica_groups=REPLICA_GROUPS,
                ins=[bounce_k_in.opt()], outs=[bounce_k_out.opt()])

    def emit_exchange_v(h):
        """Exchange half h (8 k-tiles) of the local V1 so the first half
        lands while the later V blocks are still projecting."""
        nc.sync.dma_start(
            out=bounce_v_in[h].rearrange("(p t h) -> p t h", p=128, h=2 * H1),
            in_=v1_sb[:, ds(8 * h, 8), :].bitcast(U8))
        if fake_collective:
            nc.sync.dma_start(out=bounce_v_out[h, 0], in_=bounce_v_in[h])
            nc.sync.dma_start(out=bounce_v_out[h, 1], in_=bounce_v_in[h])
        else:
            nc.gpsimd.collective_compute(
                "AllGather", mybir.AluOpType.bypass,
                replica_groups=REPLICA_GROUPS,
                ins=[bounce_v_in[h].opt()], outs=[bounce_v_out[h].opt()])

    def emit_gather_k():
        psec_reg = nc.gpsimd.alloc_register(f"psec_reg_{nc.next_id()}")
        nc.gpsimd.reg_load(psec_reg, ap["psec"][0:1, 0:1])
        psec = nc.gpsimd.snap(psec_reg, donate=True, min_val=0, max_val=1)
        nc.gpsimd.dma_start(
            out=k8_sb[:, 0, ds(TL, TL)].bitcast(U8),
            in_=bounce_k_out[ds(psec, 1), :].rearrange(
                "s (p t) -> p (s t)", p=64))

    def emit_gather_v(h):
        psec_reg = nc.gpsimd.alloc_register(f"psec_reg_{nc.next_id()}")
        nc.gpsimd.reg_load(psec_reg, ap["psec"][0:1, 0:1])
        psec = nc.gpsimd.snap(psec_reg, donate=True, min_val=0, max_val=1)
        nc.gpsimd.dma_start(
            out=v1_sb[:, ds(KTL + 8 * h, 8), :].bitcast(U8),
            in_=bounce_v_out[h, ds(psec, 1), :].rearrange(
                "s (p t h) -> p (s t) h", p=128, h=2 * H1))

    # ---- attention ----
    out_dram = ap["out"]
    pt_tiles = {}            # (c, kt) -> (P tile, col)
    # greedy earliest-finish exp assignment: modeled cumulative busy ns
    eng_t = {"act": 0.0, "dve": 0.0}

    def pt_ap(c, kt, qs):
        """P^T slice [128, 128] for (chunk c, k-tile kt, query sub qs)."""
        pt, j = pt_tiles[(c, kt)]
        return pt[:, j, ts(qs, 128)]

    def emit_score_pair_split(c, g):
        """Last score group: the two k-tiles' exps run CONCURRENTLY on
        ACT and DVE (512 elems each) so the final exp latency is halved
        and only 8 PV matmuls trail each."""
        st = psum.tile([128, 2, 512], F32, tag="st", bufs=3, name="st")
        for j in range(2):
            kt = 2 * g + j
            nc.tensor.matmul(st[:, j], k8_sb[:, :, ts(kt, 128)],
                             q8_sb[:, :, ts(c, 512)],
                             start=True, stop=True, perf_mode=DR)
        ptA = sbuf.tile([128, 1, 512], BF16, tag="ptT", bufs=2, name="ptA")
        ptD = sbuf.tile([128, 1, 512], BF16, tag="ptT", bufs=2, name="ptD")
        ASSIGN_LOG.append(True)
        nc.scalar.activation(out=ptA, in_=st[:, 0:1], func=EXP, scale=SCALE)
        nc.vector.tensor_scalar(
            out=ptD.bitcast(I16), in0=st[:, 1:2],
            scalar1=float(A_SCH), scalar2=float(B_SCH),
            op0=mybir.AluOpType.mult, op1=mybir.AluOpType.add)
        pt_tiles[(c, 2 * g)] = (ptA, 0)
        pt_tiles[(c, 2 * g + 1)] = (ptD, 0)

    def emit_score_pair(c, g):
        """k-tiles (2g, 2g+1) vs query chunk c.

        Engine chosen greedily by modeled finish time; each engine owns
        its own PSUM score pool so neither gates on the other's exp
        completions (slot-rotation lockstep): ACT gets 2-bank pair
        tiles (one 1024-elem exp), the DVE gets two 1-bank single
        tiles (two 512-elem Schraudolph ops, slightly more init but
        decoupled)."""
        st = psum.tile([128, 2, 512], F32, tag="st", bufs=3, name="st")
        for j in range(2):
            kt = 2 * g + j
            nc.tensor.matmul(st[:, j], k8_sb[:, :, ts(kt, 128)],
                             q8_sb[:, :, ts(c, 512)],
                             start=True, stop=True, perf_mode=DR)
        pt = sbuf.tile([128, 2, 512], BF16, tag="pt", bufs=64)
        if EXP_PATTERN is not None:
            use_act = EXP_PATTERN[len(ASSIGN_LOG)]
        else:
            use_act = (eng_t["act"] + ACT_EXP_NS
                       <= eng_t["dve"] + DVE_EXP_NS + DVE_BIAS)
        ASSIGN_LOG.append(bool(use_act))
        if use_act:
            eng_t["act"] += ACT_EXP_NS
            nc.scalar.activation(out=pt, in_=st, func=EXP, scale=SCALE)
        else:
            eng_t["dve"] += DVE_EXP_NS
            nc.vector.tensor_scalar(
                out=pt.bitcast(I16), in0=st, scalar1=float(A_SCH),
                scalar2=float(B_SCH),
                op0=mybir.AluOpType.mult, op1=mybir.AluOpType.add)
        for j in range(2):
            pt_tiles[(c, 2 * g + j)] = (pt, j)



    # One long-lived PSUM accumulation chain per chunk ([128, 4, 128] =
    # one bank, 4 open qs-groups): the local 16 k-tiles open the chain
    # during the local phase (when PE has slack), the remote 16 continue
    # it in the remote loop -- no oL staging copies or adds.
    o_ps = {}

    def emit_pv_local_qs(c, qs):
        """Open the chain for chunk c, query sub qs: local-half 16 MMs.

        start=True flags the chunk tile's whole 2KB zero region as
        pending-zero, so it must appear EXACTLY ONCE per chunk bank (on
        the first matmul); the other qs groups' first writes then see
        pending-zero bytes and are implicitly zeroed -- a second start
        would wipe the still-open earlier groups' partials."""
        first = c not in o_ps
        if first:
            o_ps[c] = psum.tile([128, 4, 128], F32, tag="acc65", bufs=2,
                                name=f"o{c}")
        acc = o_ps[c][:, qs, 0:H1]
        for kt in range(16):
            nc.tensor.matmul(acc, pt_ap(c, kt, qs),
                             v1_sb[:, kt, :],
                             start=(first and kt == 0), stop=False)

    def emit_pv_piece(c, qs, half):
        """8 accumulating PV matmuls: queries [128qs], k-tiles half*8+16..."""
        acc = o_ps[c][:, qs, 0:H1]
        k0, k1 = (16, 24) if half == 0 else (24, 32)
        for kt in range(k0, k1):
            nc.tensor.matmul(acc, pt_ap(c, kt, qs),
                             v1_sb[:, kt, :],
                             start=False, stop=(kt == KT - 1))

    def emit_out(c, eng="vector"):
        outf = sbuf.tile([128, 4, H1], F32, tag="outf", bufs=2)
        if eng == "vector":
            nc.vector.tensor_copy(out=outf, in_=o_ps[c][:, :, 0:H1])
            eng_t["dve"] += OUT_NS
        else:
            nc.scalar.copy(out=outf, in_=o_ps[c][:, :, 0:H1])
            eng_t["act"] += OUT_NS
        nc.sync.dma_start(out=out_dram[:, ds(4 * c, 4), :], in_=outf)
        del o_ps[c]

    # Local phase, triangular, paced by the x8 chunk DMAs (~1.1us each):
    # S(kr, qc) = the two score pair-groups of k-chunk kr vs query chunk
    # qc.  V blocks start as their ex8 chunks land (~5.5us on).
    def emit_s_block(kr, qc):
        emit_score_pair(qc, 2 * kr)
        emit_score_pair(qc, 2 * kr + 1)

    # Projections + K exchange front-loaded: the exchange DMA waits on
    # the chunk-3 k8 copy, so that copy must not queue behind exp work
    # in the in-order DVE stream.
    emit_kq_chunk(0)
    emit_s_block(0, 0)
    emit_kq_chunk(1)
    emit_s_block(1, 0)
    emit_s_block(0, 1)
    emit_kq_chunk(2)
    emit_s_block(1, 1)
    emit_kq_chunk(3)
    emit_exchange_k()
    emit_gather_k()
    emit_s_block(2, 0)
    emit_s_block(2, 1)
    emit_v_block(0)
    emit_s_block(0, 2)
    emit_s_block(1, 2)
    emit_v_block(1)
    emit_s_block(2, 2)
    emit_s_block(3, 0)
    emit_exchange_v(0)
    emit_gather_v(0)
    emit_s_block(3, 1)
    emit_v_block(2)
    emit_s_block(0, 3)
    emit_s_block(1, 3)
    emit_v_block(3)
    emit_exchange_v(1)
    emit_gather_v(1)
    emit_s_block(3, 2)
    emit_pv_local_qs(0, 0)
    emit_s_block(2, 3)
    emit_pv_local_qs(0, 1)
    emit_pv_local_qs(0, 2)
    emit_s_block(3, 3)
    emit_pv_local_qs(0, 3)
    emit_pv_local_qs(1, 0)
    emit_pv_local_qs(1, 1)
    emit_pv_local_qs(1, 2)
    emit_pv_local_qs(1, 3)

    # Phase B: remote pairs, chunk-major.  Chunk c's remote PV runs
    # kt-major two score-groups behind its own exps; chunks 2/3 open
    # their chains here (local-half PV interleaved between this chunk's
    # score groups) so at most two chains are ever live (2 PSUM banks).
    def emit_rpv_g(c, g):
        """Remote PV for score group g: k-tiles 2g, 2g+1, all 4 qs."""
        for kt in (2 * g, 2 * g + 1):
            for qs in range(4):
                nc.tensor.matmul(
                    o_ps[c][:, qs, 0:H1], pt_ap(c, kt, qs),
                    v1_sb[:, kt, :], start=False, stop=(kt == KT - 1))

    # The remote PV lags its exps by two score groups and the lag is
    # carried ACROSS chunk boundaries so the next chunk's score matmuls
    # (feeding the exp engines) are never queued behind a chunk's
    # trailing PV in the in-order PE stream.  The last chunk drops to
    # lag-1 so only ~32 matmuls trail the final exp.
    seq = [(c, g) for c in range(QC) for g in range(8, 16)]
    pv_done = 0

    def pump_rpv(upto):
        nonlocal pv_done
        while pv_done < upto:
            emit_rpv_g(*seq[pv_done])
            pv_done += 1

    for i, (c, g) in enumerate(seq):
        emit_score_pair(c, g)
        if c >= 2 and g < 10:
            # this chunk's local-half PV (pt ready long ago); all 4
            # qs groups must open before the first remote PV of c
            emit_pv_local_qs(c, 2 * (g - 8))
            emit_pv_local_qs(c, 2 * (g - 8) + 1)
        lag = 1 if (c == QC - 1 and g >= 13) else 2
        pump_rpv(i + 1 - lag)
        if g == 9 and c > 0:
            # previous chunk's out copy, emitted after two of this
            # chunk's score groups (and after its trailing PV above):
            # keeps the copy from sitting in the exp engine's in-order
            # stream ahead of this chunk's first exps
            emit_out(c - 1, eng="vector" if c % 2 == 1 else "scalar")
    pump_rpv(len(seq))
    emit_out(3, eng="scalar")
    dram_cm.__exit__(None, None, None)


def build(repeat=1, fake_collective=False, num_devices=NCORES,
          timing_mode=False):
    nc = bacc.Bacc("TRN2", target_bir_lowering=False, debug=False,
                   num_devices=num_devices)
    xT_kind = "Internal" if timing_mode else "ExternalInput"
    ap = {
        "xT8": nc.dram_tensor("xT8", [128, QC, DT // 2, 2, 512], F8,
                              kind=xT_kind).ap(),
        "xTe": nc.dram_tensor("xTe", [128, QC, DT // 2, 2, 512], F8,
                              kind=xT_kind).ap(),
        "wpack": nc.dram_tensor("wpack", [128, 2 * DT * WCOLS_P], F8,
                                kind="ExternalInput").ap(),
        "bq": nc.dram_tensor("bq", [128, 1], F32, kind="ExternalInput").ap(),
        "psec": nc.dram_tensor("psec", [1, 1], mybir.dt.uint32,
                               kind="ExternalInput").ap(),
        "out": nc.dram_tensor("out", [128, KTL, H1], F32,
                              kind="ExternalOutput").ap(),
    }
    with tile.TileContext(nc) as tc:
        with tc.tile_pool(name="psum", bufs=2, space="PSUM") as psum, \
             tc.tile_pool(name="sbuf", bufs=2) as sbuf:
            for _ in range(repeat):
                build_body(nc, tc, ap, psum, sbuf, fake_collective)
    nc.compile()
    return nc


def make_in_maps(x, Wq, bq, Wk, bk, Wv, bv):
    """Per-core input shards. bk is intentionally unused (softmax-invariant);
    bv is applied on the host (also softmax-invariant)."""
    del bk, bv
    x = np.asarray(x, np.float32)
    wqT = np.asarray(Wq, np.float32).T                      # [768, 64]
    wkT = np.asarray(Wk, np.float32).T
    wv1 = np.concatenate(
        [np.asarray(Wv, np.float32).T, np.zeros((D, 1), np.float32)], axis=1)
    wpack = np.concatenate([wqT, wkT, wv1], axis=1)       # [768, 193]
    f8 = ml_dtypes.float8_e4m3
    wpack = np.concatenate(
        [wpack, np.zeros((D, WCOLS_P - WCOLS), np.float32)], axis=1)
    w32 = (wpack * 32.0).astype(np.float32)
    w8 = w32.astype(f8)
    ew8 = (w32 - w8.astype(np.float32)).astype(f8)
    # device layout [128 p, 2 slot, DT, WCOLS] contiguous per partition
    wpack_h = np.ascontiguousarray(
        np.stack([w8.reshape(DT, 128, WCOLS_P).transpose(1, 0, 2),
                  ew8.reshape(DT, 128, WCOLS_P).transpose(1, 0, 2)], axis=1))
    bq_h = np.zeros((128, 1), np.float32)
    bq_h[0:64, 0] = np.asarray(bq, np.float32)

    in_maps = []
    for i in range(NCORES):
        b, half = i // 2, i % 2
        xh = x[b, half * TL:(half + 1) * TL, :]          # [2048, 768]
        xT_full = xh.T.astype(np.float32)                 # [768, 2048]
        x8 = xT_full.astype(f8)
        ex8 = (xT_full - x8.astype(np.float32)).astype(f8)
        # [p, c, j, d_in_pair, t] per stream (x8 / ex8)
        def lay(a):
            arr = a.reshape(DT, 128, QC, 512)             # [d, p, c, t]
            return np.ascontiguousarray(
                arr.transpose(1, 2, 0, 3).reshape(128, QC, DT // 2, 2, 512))
        in_maps.append({
            "xT8": lay(x8), "xTe": lay(ex8), "wpack": wpack_h, "bq": bq_h,
            "psec": np.array([[1 - (i % 2)]], np.uint32),
        })
    return in_maps


_NC_CACHE = {}


def kernel(x, Wq, bq, Wk, bk, Wv, bv):
    if "nc" not in _NC_CACHE:
        _NC_CACHE["nc"] = build()
    nc = _NC_CACHE["nc"]
    in_maps = make_in_maps(x, Wq, bq, Wk, bk, Wv, bv)
    res = run_bass_kernel_spmd(nc, in_maps, core_ids=list(range(NCORES)))
    bv_h = np.asarray(bv, np.float32)
    out = np.empty((B, T, H), np.float32)
    for i in range(NCORES):
        b, half = i // 2, i % 2
        r = res.results[i]["out"]                        # [128, 16, 65]
        r = r.transpose(1, 0, 2).reshape(TL, H1)         # token-major
        out[b, half * TL:(half + 1) * TL, :] = (
            r[:, 0:H] / r[:, H:H1] + bv_h)
    return out
